# revision 4
# baseline (speedup 1.0000x reference)
"""Multi-head attention (S=2048, D=1024, H=16, dk=dv=64) on 8 TRN2 NeuronCores.

Sharding: head-parallel tensor parallelism. Core c owns heads {2c, 2c+1}:
  - computes QT/KT (transposed per-head projections), V, scoresT, softmax
    (over the partition axis via a ones-column in the V matmul for the
    denominators), and the per-head context ctxT.
  - the 8 cores' ctxT blocks are AllGathered into the full concat [1024, S],
  - each core then computes a 128-column slice of the output projection
    (outT layout), so the host-side unshard is a pure concat+transpose.

Compute dtype: bf16 operands with fp32 PSUM accumulation; softmax exp/
normalization in fp32.
"""

import numpy as np

import concourse.bass as bass
import concourse.mybir as mybir
import concourse.tile as tile
from concourse import bacc
from concourse.bass_utils import run_bass_kernel_spmd

S = 2048
D = 1024
H = 16
DK = 64
DV = 64
NCORES = 8
HPC = H // NCORES          # heads per core = 2
FW = HPC * DV              # per-core feature width = 128
P = 128                    # partitions
KT_D = D // P              # 8 contraction tiles over D
TT = S // P                # 16 tiles over t (keys)
SCH = 1024                 # s-chunk width for attention
NSC = S // SCH             # 2 s-chunks
NQ = 512                   # matmul moving free dim

F32 = mybir.dt.float32
BF16 = mybir.dt.bfloat16

_cache = {}


def build(enc_ag: bool = True):
    """Build the SPMD program. enc_ag: convert enc slices to bf16 and
    AllGather them (sharded conversion) instead of each core cast-reading
    the full f32 enc tensors."""
    nc = bacc.Bacc(None, target_bir_lowering=False)

    # ---- I/O ----
    if enc_ag:
        # per-core d-slice of each transposed encoding, f32 [128, S]
        encs_in = {
            n: nc.dram_tensor(n, [P, S], F32, kind="ExternalInput")
            for n in ("encq_sl", "enck_sl", "encv_sl")
        }
    else:
        encs_in = {
            n: nc.dram_tensor(n, [D, S], F32, kind="ExternalInput")
            for n in ("encq_t", "enck_t", "encv_t")
        }
    wq = nc.dram_tensor("wq", [D, FW], F32, kind="ExternalInput")
    wk = nc.dram_tensor("wk", [D, FW], F32, kind="ExternalInput")
    wv = nc.dram_tensor("wv", [D, FW], F32, kind="ExternalInput")
    wo = nc.dram_tensor("wo", [D, FW], F32, kind="ExternalInput")
    out_t = nc.dram_tensor("outT", [FW, S], F32, kind="ExternalOutput")

    with tile.TileContext(nc) as tc:
        with (
            tc.tile_pool(name="wts", bufs=1) as wts,
            tc.tile_pool(name="encp", bufs=3) as encp,
            tc.tile_pool(name="qkv", bufs=1) as qkv,
            tc.tile_pool(name="expp", bufs=3) as expp,
            tc.tile_pool(name="catp", bufs=1) as catp,
            tc.tile_pool(name="catin", bufs=3) as catin,
            tc.tile_pool(name="misc", bufs=2) as misc,
            tc.tile_pool(name="dram", bufs=1, space="DRAM") as dram,
        ):
            rg = [list(range(NCORES))]

            # ---- weights: cast-DMA f32 -> bf16, laid out [128, KT_D, FW] ----
            wtiles = {}
            for name, t in (("wq", wq), ("wk", wk), ("wv", wv), ("wo", wo)):
                wt = wts.tile([P, KT_D, FW], BF16, tag=f"w_{name}")
                nc.gpsimd.dma_start(
                    wt[:], t.rearrange("(kt p) m -> p kt m", p=P)
                )
                wtiles[name] = wt

            # ---- encodings -> bf16 SBUF tiles [128, S] per d-tile ----
            # enc_bf[x][dt] produced either from the AllGathered bf16 copy
            # (enc_ag) or by cast-DMA straight from the full f32 input.
            if enc_ag:
                enc_gathered = {}
                for sl_name, g_name in (
                    ("encq_sl", "q"), ("enck_sl", "k"), ("encv_sl", "v")
                ):
                    sl_bf = misc.tile([P, S], BF16, tag=f"slbf_{g_name}")
                    nc.gpsimd.dma_start(sl_bf[:], encs_in[sl_name][:])
                    bounce = dram.tile([P, S], BF16, tag=f"bnc_{g_name}")
                    nc.sync.dma_start(bounce[:], sl_bf[:])
                    gath = dram.tile([D, S], BF16, tag=f"gth_{g_name}")
                    nc.gpsimd.collective_compute(
                        "AllGather",
                        mybir.AluOpType.bypass,
                        ins=[bounce[:].opt()],
                        outs=[gath[:].opt()],
                        replica_groups=rg,
                    )
                    enc_gathered[g_name] = gath

                def load_enc(x, dt, tag):
                    t = encp.tile([P, S], BF16, tag=tag)
                    nc.sync.dma_start(
                        t[:], enc_gathered[x][dt * P : (dt + 1) * P, :]
                    )
                    return t
            else:
                enc_full = {
                    "q": encs_in["encq_t"],
                    "k": encs_in["enck_t"],
                    "v": encs_in["encv_t"],
                }

                def load_enc(x, dt, tag):
                    t = encp.tile([P, S], BF16, tag=tag)
                    nc.gpsimd.dma_start(
                        t[:], enc_full[x][dt * P : (dt + 1) * P, :]
                    )
                    return t

            # ---- Q/K projections: QT/KT [128, S] bf16 (heads stacked) ----
            # dt-outer accumulation: psum accumulators held across dt loop.
            qt_sb = qkv.tile([P, S], BF16, tag="qt")
            kt_sb = qkv.tile([P, S], BF16, tag="kt")
            ps_a = tc.tile_pool(name="ps_a", bufs=1, space="PSUM")
            ps_qk = ps_a.__enter__()
            acc = {}
            for pj in ("q", "k"):
                for sc4 in range(S // NQ):
                    acc[(pj, sc4)] = ps_qk.tile([P, NQ], F32, tag=f"pqk{pj}{sc4}", name=f"pqk{pj}{sc4}")
            for dt in range(KT_D):
                eq = load_enc("q", dt, "encq")
                ek = load_enc("k", dt, "enck")
                for pj, et in (("q", eq), ("k", ek)):
                    w = wtiles[f"w{pj}"]
                    for sc4 in range(S // NQ):
                        nc.tensor.matmul(
                            acc[(pj, sc4)][:],
                            w[:, dt, :],
                            et[:, sc4 * NQ : (sc4 + 1) * NQ],
                            start=(dt == 0),
                            stop=(dt == KT_D - 1),
                        )
            for pj, sb in (("q", qt_sb), ("k", kt_sb)):
                for sc4 in range(S // NQ):
                    nc.vector.tensor_copy(
                        sb[:, sc4 * NQ : (sc4 + 1) * NQ], acc[(pj, sc4)][:]
                    )
            ps_a.__exit__(None, None, None)
            ps_v_cm = tc.tile_pool(name="ps_v", bufs=1, space="PSUM")
            ps_qk = ps_v_cm.__enter__()

            # ---- V: VT [128, S] then PE-transpose into V_aug [128, TT, 130]
            # (per head: 64 v-columns + a ones column for softmax denom) ----
            vt_acc = {
                sc4: ps_qk.tile([P, NQ], F32, tag=f"pv{sc4}", name=f"pv{sc4}")
                for sc4 in range(S // NQ)
            }
            for dt in range(KT_D):
                ev = load_enc("v", dt, "encv")
                for sc4 in range(S // NQ):
                    nc.tensor.matmul(
                        vt_acc[sc4][:],
                        wtiles["wv"][:, dt, :],
                        ev[:, sc4 * NQ : (sc4 + 1) * NQ],
                        start=(dt == 0),
                        stop=(dt == KT_D - 1),
                    )
            vt_sb = qkv.tile([P, S], BF16, tag="vt")
            for sc4 in range(S // NQ):
                nc.vector.tensor_copy(
                    vt_sb[:, sc4 * NQ : (sc4 + 1) * NQ], vt_acc[sc4][:]
                )

            v_aug = qkv.tile([P, TT, 2 * (DV + 1)], BF16, tag="vaug")
            nc.any.memset(v_aug[:, :, DV : DV + 1], 1.0)
            nc.any.memset(v_aug[:, :, 2 * DV + 1 : 2 * DV + 2], 1.0)
            ident = wts.tile([P, P], BF16, tag="ident")
            from concourse.masks import make_identity

            make_identity(nc, ident)
            for tt in range(TT):
                tp = ps_qk.tile([P, P], BF16, tag="ptr", bufs=2)
                nc.tensor.transpose(
                    tp[:], vt_sb[:, tt * P : (tt + 1) * P], ident[:]
                )
                nc.vector.tensor_copy(v_aug[:, tt, 0:DV], tp[:, 0:DV])
                nc.vector.tensor_copy(
                    v_aug[:, tt, DV + 1 : 2 * DV + 1], tp[:, DV : 2 * DV]
                )

            ps_v_cm.__exit__(None, None, None)
            ps_at_cm = tc.tile_pool(name="ps_at", bufs=1, space="PSUM")
            ps_at = ps_at_cm.__enter__()

            # ---- attention (both heads), streaming t-tiles ----
            cat_loc = catp.tile([P, S], BF16, tag="cat")
            for sc in range(NSC):
                ctx_ps = {
                    h: ps_at.tile([DV + 1, SCH], F32, tag=f"ctx{h}", name=f"ctx{h}")
                    for h in range(HPC)
                }
                for tt in range(TT):
                    sc_ps = {}
                    for h in range(HPC):
                        sc_ps[h] = ps_at.tile([P, SCH], F32, tag=f"sco{h}", name=f"sco{h}")
                        for nn in range(SCH // NQ):
                            nc.tensor.matmul(
                                sc_ps[h][:, nn * NQ : (nn + 1) * NQ],
                                kt_sb[
                                    h * DK : (h + 1) * DK,
                                    tt * P : (tt + 1) * P,
                                ],
                                qt_sb[
                                    h * DK : (h + 1) * DK,
                                    sc * SCH + nn * NQ : sc * SCH + (nn + 1) * NQ,
                                ],
                                start=True,
                                stop=True,
                            )
                    for h in range(HPC):
                        ex = expp.tile([P, SCH], BF16, tag=f"exp{h}")
                        nc.scalar.activation(
                            ex[:],
                            sc_ps[h][:],
                            mybir.ActivationFunctionType.Exp,
                            scale=1.0 / np.sqrt(DK),
                        )
                        for nn in range(SCH // NQ):
                            nc.tensor.matmul(
                                ctx_ps[h][:, nn * NQ : (nn + 1) * NQ],
                                v_aug[
                                    :, tt, h * (DV + 1) : (h + 1) * (DV + 1)
                                ],
                                ex[:, nn * NQ : (nn + 1) * NQ],
                                start=(tt == 0),
                                stop=(tt == TT - 1),
                            )
                # normalize: divide by the ones-column sums (row DV)
                for h in range(HPC):
                    recip = misc.tile([1, SCH], F32, tag="recip")
                    nc.vector.reciprocal(recip[:], ctx_ps[h][DV : DV + 1, :])
                    bcast = misc.tile([DV, SCH], F32, tag="bcast")
                    nc.gpsimd.partition_broadcast(bcast[:], recip[:])
                    nc.vector.tensor_mul(
                        cat_loc[
                            h * DV : (h + 1) * DV, sc * SCH : (sc + 1) * SCH
                        ],
                        ctx_ps[h][0:DV, :],
                        bcast[:],
                    )

            ps_at_cm.__exit__(None, None, None)
            ps_o_cm = tc.tile_pool(name="ps_o", bufs=1, space="PSUM")
            ps_qk = ps_o_cm.__enter__()

            # ---- AllGather ctxT across cores -> full concat [D, S] bf16 ----
            cat_bounce = dram.tile([P, S], BF16, tag="catb")
            nc.sync.dma_start(cat_bounce[:], cat_loc[:])
            cat_all = dram.tile([D, S], BF16, tag="catall")
            nc.gpsimd.collective_compute(
                "AllGather",
                mybir.AluOpType.bypass,
                ins=[cat_bounce[:].opt()],
                outs=[cat_all[:].opt()],
                replica_groups=rg,
            )

            # ---- output projection: outT [128, S] = wo.T @ cat_all ----
            out_acc = {
                sc4: ps_qk.tile([P, NQ], F32, tag=f"po{sc4}", name=f"po{sc4}")
                for sc4 in range(S // NQ)
            }
            for kt in range(KT_D):
                ct = catin.tile([P, S], BF16, tag="catkt")
                nc.sync.dma_start(ct[:], cat_all[kt * P : (kt + 1) * P, :])
                for sc4 in range(S // NQ):
                    nc.tensor.matmul(
                        out_acc[sc4][:],
                        wtiles["wo"][:, kt, :],
                        ct[:, sc4 * NQ : (sc4 + 1) * NQ],
                        start=(kt == 0),
                        stop=(kt == KT_D - 1),
                    )
            out_sb = catp.tile([P, S], F32, tag="outsb")
            for sc4 in range(S // NQ):
                nc.vector.tensor_copy(
                    out_sb[:, sc4 * NQ : (sc4 + 1) * NQ], out_acc[sc4][:]
                )
            nc.sync.dma_start(out_t[:], out_sb[:])
            ps_o_cm.__exit__(None, None, None)

    nc.compile()
    return nc


def kernel(
    encodings_for_q,
    encodings_for_k,
    encodings_for_v,
    W_q,
    W_k,
    W_v,
    W_out,
    _enc_ag: bool = True,
    _trace: bool = False,
):
    encodings_for_q = np.asarray(encodings_for_q, dtype=np.float32)
    encodings_for_k = np.asarray(encodings_for_k, dtype=np.float32)
    encodings_for_v = np.asarray(encodings_for_v, dtype=np.float32)
    W_q = np.asarray(W_q, dtype=np.float32)
    W_k = np.asarray(W_k, dtype=np.float32)
    W_v = np.asarray(W_v, dtype=np.float32)
    W_out = np.asarray(W_out, dtype=np.float32)

    key = ("nc", _enc_ag)
    if key not in _cache:
        _cache[key] = build(enc_ag=_enc_ag)
    nc = _cache[key]

    eqT = np.ascontiguousarray(encodings_for_q.T)
    ekT = np.ascontiguousarray(encodings_for_k.T)
    evT = np.ascontiguousarray(encodings_for_v.T)

    in_maps = []
    for c in range(NCORES):
        hs = slice(HPC * c, HPC * (c + 1))
        m = {
            "wq": np.ascontiguousarray(
                np.transpose(W_q[hs], (1, 0, 2)).reshape(D, FW)
            ),
            "wk": np.ascontiguousarray(
                np.transpose(W_k[hs], (1, 0, 2)).reshape(D, FW)
            ),
            "wv": np.ascontiguousarray(
                np.transpose(W_v[hs], (1, 0, 2)).reshape(D, FW)
            ),
            "wo": np.ascontiguousarray(W_out[:, FW * c : FW * (c + 1)]),
        }
        if _enc_ag:
            m["encq_sl"] = np.ascontiguousarray(eqT[P * c : P * (c + 1), :])
            m["enck_sl"] = np.ascontiguousarray(ekT[P * c : P * (c + 1), :])
            m["encv_sl"] = np.ascontiguousarray(evT[P * c : P * (c + 1), :])
        else:
            m["encq_t"] = eqT
            m["enck_t"] = ekT
            m["encv_t"] = evT
        in_maps.append(m)

    r = run_bass_kernel_spmd(
        nc, in_maps, core_ids=list(range(NCORES)), trace=_trace
    )
    out = np.concatenate(
        [r.results[c]["outT"].T for c in range(NCORES)], axis=1
    )
    if _trace:
        kernel.last_exec_time_ns = r.exec_time_ns
    return out.astype(np.float32)


# revision 5
# speedup vs baseline: 1.2568x; 1.2568x over previous
"""Multi-head attention (S=2048, D=1024, H=16, dk=dv=64) on 8 TRN2 NeuronCores.

Sharding: head-parallel tensor parallelism. Core c owns heads {2c, 2c+1}:
  - computes QT/KT (transposed per-head projections), V, scoresT, softmax
    (over the partition axis via a ones-column in the V matmul for the
    denominators), and the per-head context ctxT.
  - the 8 cores' ctxT blocks are AllGathered into the full concat [1024, S],
  - each core then computes a 128-column slice of the output projection
    (outT layout), so the host-side unshard is a pure concat+transpose.

Compute dtype: bf16 operands with fp32 PSUM accumulation; softmax exp/
normalization in fp32.
"""

import numpy as np

import concourse.bass as bass
import concourse.mybir as mybir
import concourse.tile as tile
from concourse import bacc
from concourse.bass_utils import run_bass_kernel_spmd

S = 2048
D = 1024
H = 16
DK = 64
DV = 64
NCORES = 8
HPC = H // NCORES          # heads per core = 2
FW = HPC * DV              # per-core feature width = 128
P = 128                    # partitions
KT_D = D // P              # 8 contraction tiles over D
TT = S // P                # 16 tiles over t (keys)
SCH = 1024                 # s-chunk width for attention
NSC = S // SCH             # 2 s-chunks
NQ = 512                   # matmul moving free dim

F32 = mybir.dt.float32
BF16 = mybir.dt.bfloat16

_cache = {}


def build(enc_ag: bool = True):
    """Build the SPMD program. enc_ag: convert enc slices to bf16 and
    AllGather them (sharded conversion) instead of each core cast-reading
    the full f32 enc tensors."""
    nc = bacc.Bacc(None, target_bir_lowering=False)

    # ---- I/O ----
    if enc_ag:
        # per-core d-slice of each transposed encoding, f32 [128, S]
        encs_in = {
            n: nc.dram_tensor(n, [P, S], F32, kind="ExternalInput")
            for n in ("encq_sl", "enck_sl", "encv_sl")
        }
    else:
        encs_in = {
            n: nc.dram_tensor(n, [D, S], F32, kind="ExternalInput")
            for n in ("encq_t", "enck_t", "encv_t")
        }
    wq = nc.dram_tensor("wq", [D, FW], F32, kind="ExternalInput")
    wk = nc.dram_tensor("wk", [D, FW], F32, kind="ExternalInput")
    wv = nc.dram_tensor("wv", [D, FW], F32, kind="ExternalInput")
    wo = nc.dram_tensor("wo", [D, FW], F32, kind="ExternalInput")
    out_t = nc.dram_tensor("outT", [FW, S], F32, kind="ExternalOutput")

    with tile.TileContext(nc) as tc:
        with (
            tc.tile_pool(name="wts", bufs=1) as wts,
            tc.tile_pool(name="encp", bufs=3) as encp,
            tc.tile_pool(name="qkv", bufs=1) as qkv,
            tc.tile_pool(name="expp", bufs=3) as expp,
            tc.tile_pool(name="catp", bufs=1) as catp,
            tc.tile_pool(name="catin", bufs=3) as catin,
            tc.tile_pool(name="misc", bufs=2) as misc,
            tc.tile_pool(name="dram", bufs=1, space="DRAM") as dram,
        ):
            rg = [list(range(NCORES))]

            # ---- weights: cast-DMA f32 -> bf16, laid out [128, KT_D, FW] ----
            wtiles = {}
            for name, t in (("wq", wq), ("wk", wk), ("wv", wv), ("wo", wo)):
                wt = wts.tile([P, KT_D, FW], BF16, tag=f"w_{name}")
                nc.gpsimd.dma_start(
                    wt[:], t.rearrange("(kt p) m -> p kt m", p=P)
                )
                wtiles[name] = wt

            # ---- encodings -> bf16 SBUF tiles [128, S] per d-tile ----
            # enc_bf[x][dt] produced either from the AllGathered bf16 copy
            # (enc_ag) or by cast-DMA straight from the full f32 input.
            if enc_ag:
                enc_gathered = {}
                for sl_name, g_name in (
                    ("encq_sl", "q"), ("enck_sl", "k"), ("encv_sl", "v")
                ):
                    sl_bf = misc.tile([P, S], BF16, tag=f"slbf_{g_name}")
                    nc.gpsimd.dma_start(sl_bf[:], encs_in[sl_name][:])
                    bounce = dram.tile([P, S], BF16, tag=f"bnc_{g_name}")
                    nc.sync.dma_start(bounce[:], sl_bf[:])
                    gath = dram.tile([D, S], BF16, tag=f"gth_{g_name}")
                    nc.gpsimd.collective_compute(
                        "AllGather",
                        mybir.AluOpType.bypass,
                        ins=[bounce[:].opt()],
                        outs=[gath[:].opt()],
                        replica_groups=rg,
                    )
                    enc_gathered[g_name] = gath

                def load_enc(x, dt, tag):
                    t = encp.tile([P, S], BF16, tag=tag)
                    nc.sync.dma_start(
                        t[:], enc_gathered[x][dt * P : (dt + 1) * P, :]
                    )
                    return t
            else:
                enc_full = {
                    "q": encs_in["encq_t"],
                    "k": encs_in["enck_t"],
                    "v": encs_in["encv_t"],
                }

                def load_enc(x, dt, tag):
                    t = encp.tile([P, S], BF16, tag=tag)
                    nc.gpsimd.dma_start(
                        t[:], enc_full[x][dt * P : (dt + 1) * P, :]
                    )
                    return t

            # ---- Q/K projections: QT/KT [128, S] bf16 (heads stacked) ----
            # dt-outer accumulation: psum accumulators held across dt loop.
            qt_sb = qkv.tile([P, S], BF16, tag="qt")
            kt_sb = qkv.tile([P, S], BF16, tag="kt")
            ps_a = tc.tile_pool(name="ps_a", bufs=1, space="PSUM")
            ps_qk = ps_a.__enter__()
            acc = {}
            for pj in ("q", "k"):
                for sc4 in range(S // NQ):
                    acc[(pj, sc4)] = ps_qk.tile([P, NQ], F32, tag=f"pqk{pj}{sc4}", name=f"pqk{pj}{sc4}")
            for dt in range(KT_D):
                eq = load_enc("q", dt, "encq")
                ek = load_enc("k", dt, "enck")
                for pj, et in (("q", eq), ("k", ek)):
                    w = wtiles[f"w{pj}"]
                    for sc4 in range(S // NQ):
                        nc.tensor.matmul(
                            acc[(pj, sc4)][:],
                            w[:, dt, :],
                            et[:, sc4 * NQ : (sc4 + 1) * NQ],
                            start=(dt == 0),
                            stop=(dt == KT_D - 1),
                        )
            for pj, sb in (("q", qt_sb), ("k", kt_sb)):
                for sc4 in range(S // NQ):
                    nc.vector.tensor_copy(
                        sb[:, sc4 * NQ : (sc4 + 1) * NQ], acc[(pj, sc4)][:]
                    )
            ps_a.__exit__(None, None, None)
            ps_v_cm = tc.tile_pool(name="ps_v", bufs=1, space="PSUM")
            ps_qk = ps_v_cm.__enter__()

            # ---- V: VT [128, S] then PE-transpose into V_aug [128, TT, 130]
            # (per head: 64 v-columns + a ones column for softmax denom) ----
            vt_acc = {
                sc4: ps_qk.tile([P, NQ], F32, tag=f"pv{sc4}", name=f"pv{sc4}")
                for sc4 in range(S // NQ)
            }
            for dt in range(KT_D):
                ev = load_enc("v", dt, "encv")
                for sc4 in range(S // NQ):
                    nc.tensor.matmul(
                        vt_acc[sc4][:],
                        wtiles["wv"][:, dt, :],
                        ev[:, sc4 * NQ : (sc4 + 1) * NQ],
                        start=(dt == 0),
                        stop=(dt == KT_D - 1),
                    )
            vt_sb = qkv.tile([P, S], BF16, tag="vt")
            for sc4 in range(S // NQ):
                nc.vector.tensor_copy(
                    vt_sb[:, sc4 * NQ : (sc4 + 1) * NQ], vt_acc[sc4][:]
                )

            v_aug = qkv.tile([P, TT, 2 * (DV + 1)], BF16, tag="vaug")
            nc.any.memset(v_aug[:, :, DV : DV + 1], 1.0)
            nc.any.memset(v_aug[:, :, 2 * DV + 1 : 2 * DV + 2], 1.0)
            ident = wts.tile([P, P], BF16, tag="ident")
            from concourse.masks import make_identity

            make_identity(nc, ident)
            for tt in range(TT):
                tp = ps_qk.tile([P, P], BF16, tag="ptr", bufs=2)
                nc.tensor.transpose(
                    tp[:], vt_sb[:, tt * P : (tt + 1) * P], ident[:]
                )
                nc.vector.tensor_copy(v_aug[:, tt, 0:DV], tp[:, 0:DV])
                nc.vector.tensor_copy(
                    v_aug[:, tt, DV + 1 : 2 * DV + 1], tp[:, DV : 2 * DV]
                )

            ps_v_cm.__exit__(None, None, None)
            ps_at_cm = tc.tile_pool(name="ps_at", bufs=1, space="PSUM")
            ps_at = ps_at_cm.__enter__()

            # ---- attention (both heads), streaming t-tiles ----
            cat_loc = catp.tile([P, S], BF16, tag="cat")
            for sc in range(NSC):
                ctx_ps = {
                    h: ps_at.tile([DV + 1, SCH], F32, tag=f"ctx{h}", name=f"ctx{h}")
                    for h in range(HPC)
                }
                for tt in range(TT):
                    sc_ps = {}
                    for h in range(HPC):
                        sc_ps[h] = ps_at.tile([P, SCH], F32, tag=f"sco{h}", name=f"sco{h}")
                        for nn in range(SCH // NQ):
                            nc.tensor.matmul(
                                sc_ps[h][:, nn * NQ : (nn + 1) * NQ],
                                kt_sb[
                                    h * DK : (h + 1) * DK,
                                    tt * P : (tt + 1) * P,
                                ],
                                qt_sb[
                                    h * DK : (h + 1) * DK,
                                    sc * SCH + nn * NQ : sc * SCH + (nn + 1) * NQ,
                                ],
                                start=True,
                                stop=True,
                            )
                    for h in range(HPC):
                        ex = expp.tile([P, SCH], BF16, tag=f"exp{h}")
                        nc.scalar.activation(
                            ex[:],
                            sc_ps[h][:],
                            mybir.ActivationFunctionType.Exp,
                            scale=1.0 / np.sqrt(DK),
                        )
                        for nn in range(SCH // NQ):
                            nc.tensor.matmul(
                                ctx_ps[h][:, nn * NQ : (nn + 1) * NQ],
                                v_aug[
                                    :, tt, h * (DV + 1) : (h + 1) * (DV + 1)
                                ],
                                ex[:, nn * NQ : (nn + 1) * NQ],
                                start=(tt == 0),
                                stop=(tt == TT - 1),
                            )
                # normalize: divide by the ones-column sums (row DV)
                for h in range(HPC):
                    recip = misc.tile([1, SCH], F32, tag="recip")
                    nc.vector.reciprocal(recip[:], ctx_ps[h][DV : DV + 1, :])
                    bcast = misc.tile([DV, SCH], F32, tag="bcast")
                    nc.gpsimd.partition_broadcast(bcast[:], recip[:])
                    nc.vector.tensor_mul(
                        cat_loc[
                            h * DV : (h + 1) * DV, sc * SCH : (sc + 1) * SCH
                        ],
                        ctx_ps[h][0:DV, :],
                        bcast[:],
                    )

            ps_at_cm.__exit__(None, None, None)
            ps_o_cm = tc.tile_pool(name="ps_o", bufs=1, space="PSUM")
            ps_qk = ps_o_cm.__enter__()

            # ---- AllGather ctxT across cores -> full concat [D, S] bf16 ----
            cat_bounce = dram.tile([P, S], BF16, tag="catb")
            nc.sync.dma_start(cat_bounce[:], cat_loc[:])
            cat_all = dram.tile([D, S], BF16, tag="catall")
            nc.gpsimd.collective_compute(
                "AllGather",
                mybir.AluOpType.bypass,
                ins=[cat_bounce[:].opt()],
                outs=[cat_all[:].opt()],
                replica_groups=rg,
            )

            # ---- output projection: outT [128, S] = wo.T @ cat_all ----
            out_acc = {
                sc4: ps_qk.tile([P, NQ], F32, tag=f"po{sc4}", name=f"po{sc4}")
                for sc4 in range(S // NQ)
            }
            for kt in range(KT_D):
                ct = catin.tile([P, S], BF16, tag="catkt")
                nc.sync.dma_start(ct[:], cat_all[kt * P : (kt + 1) * P, :])
                for sc4 in range(S // NQ):
                    nc.tensor.matmul(
                        out_acc[sc4][:],
                        wtiles["wo"][:, kt, :],
                        ct[:, sc4 * NQ : (sc4 + 1) * NQ],
                        start=(kt == 0),
                        stop=(kt == KT_D - 1),
                    )
            out_sb = catp.tile([P, S], F32, tag="outsb")
            for sc4 in range(S // NQ):
                nc.vector.tensor_copy(
                    out_sb[:, sc4 * NQ : (sc4 + 1) * NQ], out_acc[sc4][:]
                )
            nc.sync.dma_start(out_t[:], out_sb[:])
            ps_o_cm.__exit__(None, None, None)

    nc.compile()
    return nc


def kernel(
    encodings_for_q,
    encodings_for_k,
    encodings_for_v,
    W_q,
    W_k,
    W_v,
    W_out,
    _enc_ag: bool = True,
    _trace: bool = False,
):
    encodings_for_q = np.asarray(encodings_for_q, dtype=np.float32)
    encodings_for_k = np.asarray(encodings_for_k, dtype=np.float32)
    encodings_for_v = np.asarray(encodings_for_v, dtype=np.float32)
    W_q = np.asarray(W_q, dtype=np.float32)
    W_k = np.asarray(W_k, dtype=np.float32)
    W_v = np.asarray(W_v, dtype=np.float32)
    W_out = np.asarray(W_out, dtype=np.float32)

    key = ("nc", _enc_ag)
    if key not in _cache:
        _cache[key] = build(enc_ag=_enc_ag)
    nc = _cache[key]

    eqT = np.ascontiguousarray(encodings_for_q.T)
    ekT = np.ascontiguousarray(encodings_for_k.T)
    evT = np.ascontiguousarray(encodings_for_v.T)

    in_maps = []
    for c in range(NCORES):
        hs = slice(HPC * c, HPC * (c + 1))
        m = {
            "wq": np.ascontiguousarray(
                np.transpose(W_q[hs], (1, 0, 2)).reshape(D, FW)
            ),
            "wk": np.ascontiguousarray(
                np.transpose(W_k[hs], (1, 0, 2)).reshape(D, FW)
            ),
            "wv": np.ascontiguousarray(
                np.transpose(W_v[hs], (1, 0, 2)).reshape(D, FW)
            ),
            "wo": np.ascontiguousarray(W_out[:, FW * c : FW * (c + 1)]),
        }
        if _enc_ag:
            m["encq_sl"] = np.ascontiguousarray(eqT[P * c : P * (c + 1), :])
            m["enck_sl"] = np.ascontiguousarray(ekT[P * c : P * (c + 1), :])
            m["encv_sl"] = np.ascontiguousarray(evT[P * c : P * (c + 1), :])
        else:
            m["encq_t"] = eqT
            m["enck_t"] = ekT
            m["encv_t"] = evT
        in_maps.append(m)

    r = run_bass_kernel_spmd(
        nc, in_maps, core_ids=list(range(NCORES)), trace=_trace
    )
    out = np.concatenate(
        [r.results[c]["outT"].T for c in range(NCORES)], axis=1
    )
    if _trace:
        kernel.last_exec_time_ns = r.exec_time_ns
        kernel.last_insts = (
            r.instructions_and_trace[0] if r.instructions_and_trace else None
        )
    return out.astype(np.float32)


# revision 6
# speedup vs baseline: 1.7231x; 1.3710x over previous
"""Multi-head attention (S=2048, D=1024, H=16, dk=dv=64) on 8 TRN2 NeuronCores.

Sharding: head-parallel tensor parallelism. Core c owns heads {2c, 2c+1}:
  - QT/KT [128, S] (two heads stacked on partitions), V via PE-transpose of
    VT, augmented with a ones column so the ctx matmul also produces the
    softmax denominators (softmax runs over the partition axis, so the
    denominator is a matmul by ones, folded into V).
  - scoresT tiles -> exp on ACT (scale=1/8 folded in) -> ctx accumulation.
  - per s-chunk: normalize ctxT, AllGather the [128, chunk] block across
    cores -> [1024, chunk] concat, then a 128-column slice of the output
    projection per core (outT layout). Host unshard = concat + transpose.

Compute dtype: bf16 operands, fp32 PSUM accumulation, softmax in fp32.
"""

import numpy as np

import concourse.bass as bass
import concourse.mybir as mybir
import concourse.tile as tile
from concourse import bacc
from concourse.bass_utils import run_bass_kernel_spmd

S = 2048
D = 1024
H = 16
DK = 64
DV = 64
NCORES = 8
HPC = H // NCORES          # heads per core = 2
FW = HPC * DV              # per-core feature width = 128
P = 128                    # partitions
KT_D = D // P              # 8 contraction tiles over D
TT = S // P                # 16 tiles over t (keys)
SCH = 1024                 # s-chunk width for attention
NSC = S // SCH             # 2 s-chunks
NQ = 512                   # matmul moving free dim

F32 = mybir.dt.float32
BF16 = mybir.dt.bfloat16

_cache = {}


def build():
    nc = bacc.Bacc(None, target_bir_lowering=False)

    enc_in = {
        x: nc.dram_tensor(f"enc{x}_t", [D, S], F32, kind="ExternalInput")
        for x in ("q", "k", "v")
    }
    wq = nc.dram_tensor("wq", [D, FW], F32, kind="ExternalInput")
    wk = nc.dram_tensor("wk", [D, FW], F32, kind="ExternalInput")
    wv = nc.dram_tensor("wv", [D, FW], F32, kind="ExternalInput")
    wo = nc.dram_tensor("wo", [D, FW], F32, kind="ExternalInput")
    out_t = nc.dram_tensor("outT", [FW, S], F32, kind="ExternalOutput")

    with tile.TileContext(nc) as tc:
        with (
            tc.tile_pool(name="wts", bufs=1) as wts,
            tc.tile_pool(name="encp", bufs=3) as encp,
            tc.tile_pool(name="qkv", bufs=1) as qkv,
            tc.tile_pool(name="expp", bufs=6) as expp,
            tc.tile_pool(name="catp", bufs=1) as catp,
            tc.tile_pool(name="catin", bufs=3) as catin,
            tc.tile_pool(name="misc", bufs=2) as misc,
            tc.tile_pool(name="dram", bufs=1, space="DRAM") as dram,
        ):
            rg = [list(range(NCORES))]

            # ---- weights: cast-DMA f32 -> bf16, laid out [128, KT_D, FW] ----
            wtiles = {}
            for name, t in (("wq", wq), ("wk", wk), ("wv", wv), ("wo", wo)):
                wt = wts.tile([P, KT_D, FW], BF16, tag=f"w_{name}")
                nc.gpsimd.dma_start(wt[:], t.rearrange("(kt p) m -> p kt m", p=P))
                wtiles[name] = wt

            ident = wts.tile([P, P], BF16, tag="ident")
            from concourse.masks import make_identity

            make_identity(nc, ident)

            # enc d-tile loader: plain HWDGE f32 DMA + engine cast to bf16.
            # cast_eng alternates DVE / ACT to parallelize conversion.
            def load_enc(x, dt, cast_eng):
                raw = encp.tile([P, S], F32, tag=f"raw_{x}", name=f"raw_{x}")
                nc.sync.dma_start(raw[:], enc_in[x][dt * P : (dt + 1) * P, :])
                t = encp.tile([P, S], BF16, tag=f"bf_{x}", name=f"bf_{x}")
                if cast_eng == "act":
                    nc.scalar.copy(t[:], raw[:])
                else:
                    nc.vector.tensor_copy(t[:], raw[:])
                return t

            # ---- V first: VT [128, S] = wv.T @ enc_vT, then PE-transpose ----
            ps_v_cm = tc.tile_pool(name="ps_v", bufs=1, space="PSUM")
            ps_v = ps_v_cm.__enter__()
            vt_acc = {
                sc4: ps_v.tile([P, NQ], F32, tag=f"pv{sc4}", name=f"pv{sc4}")
                for sc4 in range(S // NQ)
            }
            for dt in range(KT_D):
                ev = load_enc("v", dt, "dve" if dt % 2 else "act")
                for sc4 in range(S // NQ):
                    nc.tensor.matmul(
                        vt_acc[sc4][:],
                        wtiles["wv"][:, dt, :],
                        ev[:, sc4 * NQ : (sc4 + 1) * NQ],
                        start=(dt == 0),
                        stop=(dt == KT_D - 1),
                    )
            vt_sb = qkv.tile([P, S], BF16, tag="vt")
            for sc4 in range(S // NQ):
                nc.vector.tensor_copy(
                    vt_sb[:, sc4 * NQ : (sc4 + 1) * NQ], vt_acc[sc4][:]
                )
            # V_aug [128, TT, 130]: per head 64 v-cols + ones column
            v_aug = qkv.tile([P, TT, 2 * (DV + 1)], BF16, tag="vaug")
            nc.any.memset(v_aug[:, :, DV : DV + 1], 1.0)
            nc.any.memset(v_aug[:, :, 2 * DV + 1 : 2 * DV + 2], 1.0)
            for tt in range(TT):
                tp = ps_v.tile([P, P], BF16, tag="ptr", bufs=2, name="ptr")
                nc.tensor.transpose(tp[:], vt_sb[:, tt * P : (tt + 1) * P], ident[:])
                nc.vector.tensor_copy(v_aug[:, tt, 0:DV], tp[:, 0:DV])
                nc.vector.tensor_copy(
                    v_aug[:, tt, DV + 1 : 2 * DV + 1], tp[:, DV : 2 * DV]
                )
            ps_v_cm.__exit__(None, None, None)

            # ---- Q/K projections: QT/KT [128, S] bf16 ----
            ps_a_cm = tc.tile_pool(name="ps_a", bufs=1, space="PSUM")
            ps_a = ps_a_cm.__enter__()
            qt_sb = qkv.tile([P, S], BF16, tag="qt")
            kt_sb = qkv.tile([P, S], BF16, tag="kt")
            acc = {}
            for pj in ("q", "k"):
                for sc4 in range(S // NQ):
                    acc[(pj, sc4)] = ps_a.tile(
                        [P, NQ], F32, tag=f"pqk{pj}{sc4}", name=f"pqk{pj}{sc4}"
                    )
            for dt in range(KT_D):
                eq = load_enc("q", dt, "act" if dt % 2 else "dve")
                ek = load_enc("k", dt, "dve" if dt % 2 else "act")
                for pj, et in (("q", eq), ("k", ek)):
                    w = wtiles[f"w{pj}"]
                    for sc4 in range(S // NQ):
                        nc.tensor.matmul(
                            acc[(pj, sc4)][:],
                            w[:, dt, :],
                            et[:, sc4 * NQ : (sc4 + 1) * NQ],
                            start=(dt == 0),
                            stop=(dt == KT_D - 1),
                        )
            for pj, sb in (("q", qt_sb), ("k", kt_sb)):
                for sc4 in range(S // NQ):
                    nc.vector.tensor_copy(
                        sb[:, sc4 * NQ : (sc4 + 1) * NQ], acc[(pj, sc4)][:]
                    )
            ps_a_cm.__exit__(None, None, None)

            # ---- attention: scoresT -> exp -> ctx accumulation ----
            # mega psum tile [128, SCH] = h0's 512 | h1's 512 for one half
            # chunk; one exp per mega tile. ctx accumulators [65, SCH]/head.
            ps_at_cm = tc.tile_pool(name="ps_at", bufs=1, space="PSUM")
            ps_at = ps_at_cm.__enter__()
            cat_loc = catp.tile([P, S], BF16, tag="cat")
            gath = []
            for sc in range(NSC):
                ctx_ps = {
                    h: ps_at.tile(
                        [DV + 1, SCH], F32, tag=f"ctx{h}", name=f"ctx{h}"
                    )
                    for h in range(HPC)
                }
                for tt in range(TT):
                    megas = []
                    for half in range(SCH // NQ):
                        m = ps_at.tile(
                            [P, SCH], F32, tag="mega", bufs=2, name="mega"
                        )
                        s0 = sc * SCH + half * NQ
                        for h in range(HPC):
                            nc.tensor.matmul(
                                m[:, h * NQ : (h + 1) * NQ],
                                kt_sb[h * DK : (h + 1) * DK, tt * P : (tt + 1) * P],
                                qt_sb[h * DK : (h + 1) * DK, s0 : s0 + NQ],
                                start=True,
                                stop=True,
                            )
                        megas.append(m)
                    exs = []
                    for half, m in enumerate(megas):
                        ex = expp.tile([P, SCH], BF16, tag=f"exp{half}", name="ex")
                        nc.scalar.activation(
                            ex[:],
                            m[:],
                            mybir.ActivationFunctionType.Exp,
                            scale=1.0 / np.sqrt(DK),
                        )
                        exs.append(ex)
                    for h in range(HPC):
                        for half, ex in enumerate(exs):
                            nc.tensor.matmul(
                                ctx_ps[h][:, half * NQ : (half + 1) * NQ],
                                v_aug[:, tt, h * (DV + 1) : (h + 1) * (DV + 1)],
                                ex[:, h * NQ : (h + 1) * NQ],
                                start=(tt == 0),
                                stop=(tt == TT - 1),
                            )
                # normalize by the ones-column sums (row DV), write cat block
                for h in range(HPC):
                    recip = misc.tile([1, SCH], F32, tag="recip", name="recip")
                    nc.vector.reciprocal_approx_fast(
                        recip[:], ctx_ps[h][DV : DV + 1, :]
                    )
                    bcast = misc.tile([DV, SCH], F32, tag="bcast", name="bcast")
                    nc.gpsimd.partition_broadcast(bcast[:], recip[:])
                    nc.vector.tensor_mul(
                        cat_loc[h * DV : (h + 1) * DV, sc * SCH : (sc + 1) * SCH],
                        ctx_ps[h][0:DV, :],
                        bcast[:],
                    )
                # per-chunk AllGather (overlaps the next chunk's attention)
                cb = dram.tile([P, SCH], BF16, tag=f"catb{sc}", name="cb")
                nc.sync.dma_start(cb[:], cat_loc[:, sc * SCH : (sc + 1) * SCH])
                ga = dram.tile([D, SCH], BF16, tag=f"catall{sc}", name="ga")
                nc.gpsimd.collective_compute(
                    "AllGather",
                    mybir.AluOpType.bypass,
                    ins=[cb[:].opt()],
                    outs=[ga[:].opt()],
                    replica_groups=rg,
                )
                gath.append(ga)
            ps_at_cm.__exit__(None, None, None)

            # ---- output projection per chunk: outT slice = wo.T @ cat ----
            ps_o_cm = tc.tile_pool(name="ps_o", bufs=1, space="PSUM")
            ps_o = ps_o_cm.__enter__()
            out_sb = catp.tile([P, S], F32, tag="outsb")
            for sc in range(NSC):
                oacc = {
                    nn: ps_o.tile([P, NQ], F32, tag=f"po{nn}", name=f"po{nn}")
                    for nn in range(SCH // NQ)
                }
                for kt in range(KT_D):
                    ct = catin.tile([P, SCH], BF16, tag="catkt", name="ct")
                    nc.sync.dma_start(ct[:], gath[sc][kt * P : (kt + 1) * P, :])
                    for nn in range(SCH // NQ):
                        nc.tensor.matmul(
                            oacc[nn][:],
                            wtiles["wo"][:, kt, :],
                            ct[:, nn * NQ : (nn + 1) * NQ],
                            start=(kt == 0),
                            stop=(kt == KT_D - 1),
                        )
                for nn in range(SCH // NQ):
                    off = sc * SCH + nn * NQ
                    nc.vector.tensor_copy(
                        out_sb[:, off : off + NQ], oacc[nn][:]
                    )
                nc.sync.dma_start(
                    out_t[:, sc * SCH : (sc + 1) * SCH],
                    out_sb[:, sc * SCH : (sc + 1) * SCH],
                )
            ps_o_cm.__exit__(None, None, None)

    nc.compile()
    return nc


def kernel(
    encodings_for_q,
    encodings_for_k,
    encodings_for_v,
    W_q,
    W_k,
    W_v,
    W_out,
    _trace: bool = False,
):
    encodings_for_q = np.asarray(encodings_for_q, dtype=np.float32)
    encodings_for_k = np.asarray(encodings_for_k, dtype=np.float32)
    encodings_for_v = np.asarray(encodings_for_v, dtype=np.float32)
    W_q = np.asarray(W_q, dtype=np.float32)
    W_k = np.asarray(W_k, dtype=np.float32)
    W_v = np.asarray(W_v, dtype=np.float32)
    W_out = np.asarray(W_out, dtype=np.float32)

    if "nc" not in _cache:
        _cache["nc"] = build()
    nc = _cache["nc"]

    eqT = np.ascontiguousarray(encodings_for_q.T)
    ekT = np.ascontiguousarray(encodings_for_k.T)
    evT = np.ascontiguousarray(encodings_for_v.T)

    in_maps = []
    for c in range(NCORES):
        hs = slice(HPC * c, HPC * (c + 1))
        in_maps.append(
            {
                "encq_t": eqT,
                "enck_t": ekT,
                "encv_t": evT,
                "wq": np.ascontiguousarray(
                    np.transpose(W_q[hs], (1, 0, 2)).reshape(D, FW)
                ),
                "wk": np.ascontiguousarray(
                    np.transpose(W_k[hs], (1, 0, 2)).reshape(D, FW)
                ),
                "wv": np.ascontiguousarray(
                    np.transpose(W_v[hs], (1, 0, 2)).reshape(D, FW)
                ),
                "wo": np.ascontiguousarray(W_out[:, FW * c : FW * (c + 1)]),
            }
        )

    r = run_bass_kernel_spmd(
        nc, in_maps, core_ids=list(range(NCORES)), trace=_trace
    )
    out = np.concatenate(
        [r.results[c]["outT"].T for c in range(NCORES)], axis=1
    )
    if _trace:
        kernel.last_exec_time_ns = r.exec_time_ns
        kernel.last_insts = (
            r.instructions_and_trace[0] if r.instructions_and_trace else None
        )
    return out.astype(np.float32)
